# revision 52
# baseline (speedup 1.0000x reference)
"""CoDAConv2d Trainium2 kernel (8-core SPMD, data-parallel over batch x H-halves).

Reference computation (per pixel, per sample):
    raw[o]   = w_pred[o, :] @ x + b_pred[o]          o = p*16 + co, p in [0,72)
    act[co]  = sum_p patches[p] * raw[p*16+co]
    n2[co]   = sum_p raw[p*16+co]^2
    out[co]  = act[co] / (sqrt(n2[co]) + 1e-6)

Device reformulation (never materializes the [B,72,16,H,W] weightings):
    act[co]  = sum_cp V[(cp,co)] * x[cp] + T[co]
        V    = W2^T @ patches      (static 3x3 conv, K=72 contraction on PE)
        T    = Tw^T @ patches      (PSUM-accumulated with the selection reduce)
    n2[co]   = sum_j (Y[(j,co)] + m[(j,co)])^2 + delta[co]
        Y    = Grep^T @ xrep       (K=128 on the replicated x; Grep = G/16)
        +m   is the ACT Square bias, +delta is the ACT Sqrt bias
    out      = act * reciprocal(sqrt(n2 + delta))    (eps folded; |err| ~ 6e-6)

All matmuls run in float32r (same fp32 bits; PE streams 1 cycle/col instead
of 4 — HW-verified rel err ~2e-4). f32r matmuls must write PSUM partition 0,
so the per-chunk act/norm reductions land in [16, CH] tiles and are packed
on the SBUF side instead: a DVE copy packs act into [128, CH] (4 chunks x 32
partitions), the ACT Sqrt packs the norm the same way for free. The packed
reciprocal runs once per 4-chunk pack on DVE and the final multiply runs on
the otherwise-idle GPSIMD (SBUF-only operands).

Host-side data-layout prep per core, packed per 4-chunk pack into ONE DMA:
pk [128, 2*PCH] = [ xrep (x replicated 16x over partitions) | pat (im2col,
rows 0:72) ]. DMA count is minimal (1 weights + 4 pack loads + 4 packed
stores the host unscrambles) since each dma_start costs ~1.7us of queue time.
"""

import numpy as np
from contextlib import ExitStack

C_IN = 8
C_OUT = 16
PATCH = 72          # C_IN * 3 * 3
B = 4
H = W = 112
HALF = 56           # output rows per shard (2 shards per batch sample)
NPX = HALF * W      # 6272 output pixels per core
CH = 448            # chunk = 4 output rows (matmul N, <=512 fp32 / psum bank)
NCHUNK = NPX // CH  # 14
PACK = 4            # chunks per normalize pack
NPACKS = (NCHUNK + PACK - 1) // PACK
PCH = PACK * CH     # pixels per pack
WCOLS = 322         # 128 w2 | 128 grep | 32 tw | 32 sel | 1 mv | 1 dv
NCORES = 8

_CACHE = {}


def _build_program():
    """Build + compile the per-core Bass program (same program on all cores)."""
    if "nc" in _CACHE:
        return _CACHE["nc"]
    import concourse.bacc as bacc
    import concourse.tile as tile
    from concourse import mybir

    f32 = mybir.dt.float32
    f32r = mybir.dt.float32r   # same bits; PE streams 1 cyc/col vs 4 for f32
    AF = mybir.ActivationFunctionType

    nc = bacc.Bacc("TRN2", target_bir_lowering=False, debug=False,
                   num_devices=NCORES)
    pk_d = nc.declare_dram_parameter("pk", [NPACKS * 128, 2 * PCH], f32r,
                                     isOutput=False)
    wts_d = nc.declare_dram_parameter("wts", [128, WCOLS], f32r, isOutput=False)
    out_d = nc.declare_dram_parameter("out", [NPACKS * 128, CH], f32,
                                      isOutput=True)

    with tile.TileContext(nc) as tc, ExitStack() as ctx:
        singles = ctx.enter_context(tc.tile_pool(name="singles", bufs=1))
        sb = ctx.enter_context(tc.tile_pool(name="sb", bufs=6))
        packsb = ctx.enter_context(tc.tile_pool(name="packsb", bufs=4))
        psv = ctx.enter_context(tc.tile_pool(name="psv", bufs=2, space="PSUM"))
        psy = ctx.enter_context(tc.tile_pool(name="psy", bufs=2, space="PSUM"))
        psa = ctx.enter_context(tc.tile_pool(name="psa", bufs=2, space="PSUM"))
        psn = ctx.enter_context(tc.tile_pool(name="psn", bufs=2, space="PSUM"))

        # pk0 issued first: chunk 0 compute depends on it plus the small
        # weights tile; everything else streams in behind
        pk_t = []
        pt0 = singles.tile([128, 2 * PCH], f32r, tag="pk0", name="pk0")
        nc.sync.dma_start(out=pt0[:], in_=pk_d[0:128, :])
        pk_t.append(pt0)

        wts_sb = singles.tile([128, WCOLS], f32r)
        nc.gpsimd.dma_start(out=wts_sb[:], in_=wts_d[:])
        w2_sb = wts_sb[0:PATCH, 0:128]
        gr_sb = wts_sb[0:128, 128:256]
        tw_sb = wts_sb[0:PATCH, 256:288]
        sel_sb = wts_sb[0:128, 288:320]
        mv_sb = wts_sb[0:128, 320:321]
        dv_sb = wts_sb[0:32, 321:322]

        for p in range(1, NPACKS):
            pt = singles.tile([128, 2 * PCH], f32r, tag=f"pk{p}")
            eng = nc.sync if p % 2 == 0 else nc.gpsimd
            eng.dma_start(out=pt[:], in_=pk_d[128 * p:128 * (p + 1), :])
            pk_t.append(pt)

        packs = {}
        for i in range(NCHUNK):
            p, k = i // PACK, i % PACK
            kp = min(PACK, NCHUNK - p * PACK)
            if k == 0:
                packs[p] = (
                    packsb.tile([128, CH], f32, tag="actp", name="act_pack"),
                    packsb.tile([128, CH], f32, tag="nrmp", name="nrm_pack"),
                )
            act_pack, nrm_pack = packs[p]
            lo = k * CH
            xrep = pk_t[p][:, lo:lo + CH]
            patches = pk_t[p][0:PATCH, PCH + lo:PCH + lo + CH]

            v_ps = psv.tile([128, CH], f32, tag="v")
            nc.tensor.matmul(v_ps[:], w2_sb, patches, start=True, stop=True)
            y_ps = psy.tile([128, CH], f32, tag="y")
            nc.tensor.matmul(y_ps[:], gr_sb, xrep, start=True, stop=True)

            ysq = sb.tile([128, CH], f32r, tag="ysq")
            nc.scalar.activation(ysq[:], y_ps[:], AF.Square,
                                 bias=mv_sb, scale=1.0)
            prod = sb.tile([128, CH], f32r, tag="prod")
            nc.vector.tensor_mul(prod[:], v_ps[:], xrep)

            sa_ps = psa.tile([32, CH], f32, tag="sa")
            nc.tensor.matmul(sa_ps[:], tw_sb, patches, start=True, stop=False)
            nc.tensor.matmul(sa_ps[:], sel_sb, prod[:], start=False, stop=True)
            sn_ps = psn.tile([32, CH], f32, tag="sn")
            nc.tensor.matmul(sn_ps[:], sel_sb, ysq[:], start=True, stop=True)

            sl = slice(32 * k, 32 * k + 32)
            nc.vector.tensor_copy(act_pack[sl, :], sa_ps[:])
            nc.scalar.activation(nrm_pack[sl, :], sn_ps[:], AF.Sqrt,
                                 bias=dv_sb, scale=1.0)

            if k == kp - 1:
                nprt = 32 * kp
                recip = packsb.tile([128, CH], f32, tag="recip")
                nc.vector.reciprocal(recip[:nprt], nrm_pack[:nprt])
                out_sb = packsb.tile([128, CH], f32, tag="out")
                nc.gpsimd.tensor_mul(out_sb[:nprt], act_pack[:nprt],
                                     recip[:nprt])
                nc.sync.dma_start(out=out_d[128 * p:128 * p + nprt, :],
                                  in_=out_sb[:nprt])

    nc.compile()
    _CACHE["nc"] = nc
    return nc


def make_weights(w_pred, b_pred):
    """Host-side static weight prep packed into one [128, WCOLS] fp32 array."""
    w_pred = np.asarray(w_pred, dtype=np.float64)
    b_pred = np.asarray(b_pred, dtype=np.float64)
    wr = w_pred.reshape(PATCH, C_OUT, C_IN)        # [p, co, c]
    bm = b_pred.reshape(PATCH, C_OUT)              # [p, co]
    w2 = np.ascontiguousarray(wr.transpose(0, 2, 1)).reshape(
        PATCH, C_IN * C_OUT)                       # [p, (cp,co)]
    A = np.einsum('poc,pod->ocd', wr, wr)          # [co, 8, 8]
    u = np.einsum('po,poc->oc', bm, wr)            # [co, 8]
    s = np.einsum('po,po->o', bm, bm)              # [co]
    L = np.linalg.cholesky(A)                      # [co, 8, 8]
    gq = L.transpose(1, 2, 0).reshape(C_IN, C_IN * C_OUT)  # [c, (j,co)]
    # Grep[(cp,co2), (j,co)] = G[cp, (j,co)] / 16  (sums over co2 to G @ x)
    grep = np.repeat(gq / C_OUT, C_OUT, axis=0)    # [128, 128]
    m = np.stack([np.linalg.solve(L[o], u[o]) for o in range(C_OUT)])  # [co, j]
    delta = s - (m * m).sum(1)                     # [co]
    wts = np.zeros((128, WCOLS), dtype=np.float64)
    wts[0:PATCH, 0:128] = w2
    wts[0:128, 128:256] = grep
    wts[0:PATCH, 256:272] = bm                     # tw (cols 272:288 stay 0)
    wts[0:128, 288:304] = np.tile(np.eye(C_OUT), (C_IN, 1))  # sel (cp,co)
    wts[0:128, 320] = m.T.reshape(128)             # mv: (j,co) order
    wts[0:C_OUT, 321] = delta                      # dv (rows 16:32 pad)
    wts[C_OUT:32, 321] = 1.0
    return np.ascontiguousarray(wts, dtype=np.float32)


def make_shard_inputs(in_tensor, core):
    """Host prep for one core: pk [NPACKS*128, 2*PCH] = [xrep | pat]."""
    b, sgn = core // 2, core % 2
    r0 = sgn * HALF
    pad = np.zeros((C_IN, H + 2, W + 2), dtype=np.float32)
    pad[:, 1:1 + H, 1:1 + W] = in_tensor[b]
    pat = np.empty((C_IN, 3, 3, HALF, W), dtype=np.float32)
    for di in range(3):
        for dj in range(3):
            pat[:, di, dj] = pad[:, r0 + di:r0 + di + HALF, dj:dj + W]
    pat = pat.reshape(PATCH, NPX)
    xin = in_tensor[b, :, r0:r0 + HALF, :].reshape(C_IN, NPX)
    xr16 = np.repeat(xin, C_OUT, axis=0)           # [(cp,co), n]
    pk = np.zeros((NPACKS, 128, 2 * PCH), dtype=np.float32)
    for p in range(NPACKS):
        c0, c1 = p * PCH, min(NPX, (p + 1) * PCH)
        pk[p, :, 0:c1 - c0] = xr16[:, c0:c1]
        pk[p, 0:PATCH, PCH:PCH + c1 - c0] = pat[:, c0:c1]
    return np.ascontiguousarray(pk.reshape(NPACKS * 128, 2 * PCH))


def unscramble(raw):
    """Device out [NPACKS*128, CH] -> [C_OUT, HALF, W]."""
    v = raw.reshape(NPACKS * PACK, 32, CH)[:NCHUNK, :C_OUT, :]  # [i, co, j]
    v = v.transpose(1, 0, 2).reshape(C_OUT, NPX)
    return v.reshape(C_OUT, HALF, W)


def kernel(in_tensor, w_pred, b_pred):
    from concourse.bass_utils import run_bass_kernel_spmd

    in_tensor = np.asarray(in_tensor, dtype=np.float32)
    nc = _build_program()
    wts = make_weights(w_pred, b_pred)
    in_maps = [{"pk": make_shard_inputs(in_tensor, c), "wts": wts}
               for c in range(NCORES)]
    res = run_bass_kernel_spmd(nc, in_maps, list(range(NCORES)))
    out = np.empty((B, C_OUT, H, W), dtype=np.float32)
    for c in range(NCORES):
        b, sgn = c // 2, c % 2
        out[b, :, sgn * HALF:(sgn + 1) * HALF, :] = \
            unscramble(res.results[c]["out"])
    return out
